# revision 30
# baseline (speedup 1.0000x reference)
"""DigitCaps dynamic-routing kernel for 8x Trainium2 NeuronCores.

Full inputs -> batch-sharded across 8 cores (16 samples/core), W replicated.

Per-core layout:
  u_hat[(r_l,b)=128 partitions, g=256, o=16, c=10]  (bf16 in SBUF)
    where route r = g*8 + r_l   (8 routes per matmul group)
  b_ij / c_ij: (128p, 256g, 10c)   (partition carries (r_l, b))

u_hat build (one K=128 matmul per PAIR of groups):
  lhsT = xstk[pair]  (K=128 = two stacked 64-row x-blocks, M=128=(r_l,b))
  rhs  = wd[pair]    (K=128, N=320) block-diagonal W: rows 0:64 feed cols
         0:160 (group 2m), rows 64:128 feed cols 160:320 (group 2m+1);
         zero blocks memset once in two ping-pong SBUF slots, the DMA
         rewrites only the same diagonal windows each reuse.
  out  = psum (128, 320) -> 2 MMs share a 2-bank psum tile, drained as one
         FD=640 cast to bf16, alternating scalar/vector engines.

Iteration-1 sums (c == 0.1 exactly) are PE matmuls with lhsT =
tile(eye(16),(8,8)) interleaved into the phase-1 instruction stream so the
PE queue stays dense (sums over partitions (r_l) while replicating the
result to all 128 partitions).

Iterations 2..3: fused per-chunk pipeline: agreement = DVE broadcast
multiply + tree-reduce over o, chunk-local softmax over c (exp on ACT),
c-weighted products, PE group-triple sums.
"""

import sys

for p in ("/opt/trn_rl_repo",):
    if p not in sys.path:
        sys.path.insert(0, p)

import numpy as np
import ml_dtypes

import concourse.bass as bass
import concourse.bacc as bacc
import concourse.mybir as mybir
import concourse.tile as tile
from concourse.bass_utils import run_bass_kernel_spmd

# Problem constants (hardcoded per contract)
B_FULL = 128
N_CORES = 8
B = B_FULL // N_CORES  # 16 samples per core
R = 2048
C = 10
O = 16
I = 8
ITERS = 3

RG = 8               # routes per matmul group
G = R // RG          # 256 groups
K = RG * I           # 64 contraction rows per group
CO = C * O           # 160
NP = G // 2          # 128 group pairs
SC = 8               # pairs per phase-1 slot chunk
NSC = NP // SC       # 16 slot chunks
CH = 32              # groups per routing chunk
NCH = G // CH        # 8 chunks
CPAD = 12            # padded capsule dim for 4B alignment of bf16 rows

F32 = mybir.dt.float32
BF16 = mybir.dt.bfloat16

_COMPILED = None  # cache nc across calls


def _host_prep(x, W):
    """Build per-core DMA-ready arrays. x: (128,2048,8) W: (2048,10,16,8)."""
    x = np.ascontiguousarray(x, dtype=np.float32)
    W = np.ascontiguousarray(W, dtype=np.float32)

    # Wt[g, r_l*8+i, o*10+c] = W[g*8+r_l, c, o, i]
    Wt = W.transpose(0, 3, 2, 1).reshape(G, RG, I, O, C).reshape(G, K, CO)
    # wsrc[64*gam + krow, pair, n] = Wt[2*pair+gam, krow, n]
    wsrc = np.ascontiguousarray(
        Wt.reshape(NP, 2, K, CO).transpose(1, 2, 0, 3).reshape(2 * K, NP, CO)
    ).astype(ml_dtypes.bfloat16)

    # Bmask[(r_l,b), (r_l',b')] = 1 if b==b'  -> psum = sum over r_l,
    # replicated across all output partitions
    bmask = np.tile(np.eye(B, dtype=np.float32), (RG, RG)).astype(
        ml_dtypes.bfloat16)

    in_maps = []
    for ci in range(N_CORES):
        xs = x[ci * B : (ci + 1) * B]  # (16, 2048, 8)
        # xt[g, r_l, i, b] = xs[b, g*8+r_l, i]
        xt = xs.transpose(1, 2, 0).reshape(G, RG, I, B)
        # Block-diagonal lhsT: xblk[g, r_l*8+i, r_l*16+b] = xt[g, r_l, i, b]
        xblk = np.zeros((G, RG, I, RG, B), dtype=np.float32)
        idx = np.arange(RG)
        xblk[:, idx, :, idx, :] = xt.transpose(1, 0, 2, 3)
        xblk = xblk.reshape(G, K, RG * B)
        # xstk[64*gam + krow, pair, m] = xblk[2*pair+gam, krow, m]
        xstk = np.ascontiguousarray(
            xblk.reshape(NP, 2, K, RG * B).transpose(1, 2, 0, 3)
            .reshape(2 * K, NP, RG * B)
        ).astype(ml_dtypes.bfloat16)
        # Dense x for the direct s1 = 0.1*sum_{r,i} W*x matmuls:
        # xd[64*gam + 8*rl + i, q, b] = x[b, 16*q + 8*gam + rl, i]
        xd = np.ascontiguousarray(
            xt.reshape(NP, 2, RG, I, B).transpose(1, 2, 3, 0, 4)
            .reshape(2 * K, NP, B)
        ).astype(ml_dtypes.bfloat16)
        in_maps.append({"xstk": xstk, "xd": xd, "wsrc": wsrc,
                        "bmask": bmask})
    return in_maps


def _build_kernel():
    nc = bacc.Bacc("TRN2", target_bir_lowering=False, debug=False,
                   num_devices=N_CORES)

    xstk_d = nc.dram_tensor("xstk", [2 * K, NP, RG * B], BF16,
                            kind="ExternalInput")
    xd_d = nc.dram_tensor("xd", [2 * K, NP, B], BF16, kind="ExternalInput")
    wsrc_d = nc.dram_tensor("wsrc", [2 * K, NP, CO], BF16,
                            kind="ExternalInput")
    bmask_d = nc.dram_tensor("bmask", [128, 128], BF16, kind="ExternalInput")
    vout_d = nc.dram_tensor("vout", [B, O, C], F32, kind="ExternalOutput")

    with tile.TileContext(nc) as tc:
        with (
            tc.tile_pool(name="persist", bufs=1) as persist,
            tc.tile_pool(name="xw", bufs=2) as xw,
            tc.tile_pool(name="work", bufs=7) as work,
            tc.tile_pool(name="psum", bufs=3, space="PSUM") as psum,
            tc.tile_pool(name="spsum", bufs=2, space="PSUM") as spsum,
        ):
            uhat = persist.tile([128, G, O, C], BF16)      # 80 KiB/part
            wd = [persist.tile([128, SC, 2 * CO], BF16, name=f"wd{s}")
                  for s in range(2)]                       # 2x 5 KiB
            xdt = persist.tile([128, NP, B], BF16)         # 4 KiB
            bij = persist.tile([128, G, C], BF16)          # 5 KiB
            cexp = persist.tile([128, G, C], F32)          # 10 KiB
            cbf = persist.tile([128, G, CPAD], BF16)       # 6 KiB
            bmask = persist.tile([128, 128], BF16)
            v_bf = persist.tile([128, O, C], BF16)
            s_sb = persist.tile([128, O, C], F32)
            sq = persist.tile([128, C], F32)
            sq2 = persist.tile([128, C], F32)
            zsum = persist.tile([128, G], F32)
            eps_t = persist.tile([128, 1], F32)
            nc.gpsimd.memset(eps_t[:], 1e-8)
            nc.gpsimd.memset(wd[0][:], 0.0)
            nc.gpsimd.memset(wd[1][:], 0.0)
            nc.sync.dma_start(bmask[:], bmask_d[:])

            # ---------- Phase 1: u_hat build + iteration-1 sums ----------
            # One K=128 matmul per pair of groups; 2 MMs share a 2-bank
            # psum tile drained as a single FD=640 cast. Iteration-1's
            # s1 = 0.1 * sum_r u_hat (c == 0.1 exactly) accumulates
            # directly from x and W: per pair q, two K=64 matmuls whose
            # rhs are the dense diagonal windows of the SAME wd slot the
            # u_hat matmul streams, with a 16-column stationary (xd) --
            # no extra DMA and no dependency on the u_hat drains.
            nc.sync.dma_start(xdt[:], xd_d[:])
            sp1 = spsum.tile([128, 512], F32, tag="sp", name="sp1")
            # Prime the PE's HAM clock gate: ~3.2us of matmul-busy on the
            # zeroed wd[1] slot flips the PE to 2.4 GHz before the real
            # matmuls start (LDWEIGHTS does not count as PE-busy, so the
            # LDW-heavy phase-1 stream never warms up on its own).
            for pw in range(12):
                nc.tensor.matmul(
                    sp1[:, 0 : 2 * CO], lhsT=bmask[:], rhs=wd[1][:, 0],
                    start=True, stop=True, skip_group_check=True)
            for sc in range(NSC):
                w = wd[sc % 2]
                nc.sync.dma_start(
                    w[0:K, :, 0:CO], wsrc_d[0:K, sc * SC : (sc + 1) * SC])
                nc.gpsimd.dma_start(
                    w[K : 2 * K, :, CO : 2 * CO],
                    wsrc_d[K : 2 * K, sc * SC : (sc + 1) * SC])
                xt = xw.tile([128, SC, RG * B], BF16, tag="xt")
                eng = nc.sync if sc % 2 == 0 else nc.gpsimd
                eng.dma_start(xt[:], xstk_d[:, sc * SC : (sc + 1) * SC])
                for jj in range(SC // 2):
                    ps = psum.tile([128, 2, 512], F32, tag="ps",
                                   name=f"ps{sc}_{jj}")
                    for u in range(2):
                        j = 2 * jj + u
                        nc.tensor.matmul(
                            ps[:, u, 0 : 2 * CO],
                            lhsT=xt[:, j], rhs=w[:, j],
                            start=True, stop=True)
                        # s1 += x.W for the pair's two groups: full K=128
                        # against the diag slot halves -- the zero blocks
                        # absorb the other half's rows
                        q = sc * SC + j
                        nc.tensor.matmul(
                            sp1[0:B, 0:CO], lhsT=xdt[:, q],
                            rhs=w[:, j, 0:CO],
                            start=(q == 0), stop=False,
                            skip_group_check=True)
                        nc.tensor.matmul(
                            sp1[0:B, 0:CO], lhsT=xdt[:, q],
                            rhs=w[:, j, CO : 2 * CO],
                            start=False, stop=(q == NP - 1),
                            skip_group_check=True)
                    g0 = sc * 2 * SC + 4 * jj
                    dst = uhat[:, g0 : g0 + 4].rearrange(
                        "p (x a) o c -> p x (a o c)", x=2)
                    src = ps[:, :, 0 : 2 * CO]
                    if jj % 2 == 0:
                        nc.scalar.copy(dst, src)
                    else:
                        nc.vector.tensor_copy(dst, src)
            # s1 = 0.1 * sp1; squash on the 16 valid partitions, then
            # replicate v to all 128 partitions with a bmask matmul
            nc.scalar.activation(
                s_sb[0:B].rearrange("p o c -> p (o c)"), sp1[0:B, 0:CO],
                mybir.ActivationFunctionType.Copy, scale=0.1)
            _squash(nc, work, s_sb[0:B], sq[0:B], sq2[0:B], v_bf[0:B],
                    eps_t)
            vps = spsum.tile([128, 512], F32, tag="sp", name="vps")
            nc.tensor.matmul(
                vps[:, 0:CO], lhsT=bmask[0:B],
                rhs=v_bf[0:B].rearrange("p o c -> p (o c)"),
                start=True, stop=True)
            nc.vector.tensor_copy(
                v_bf[:].rearrange("p o c -> p (o c)"), vps[:, 0:CO])

            # ---------- iterations 2..3 ----------
            # Fused per-chunk pipeline: agreement, chunk-local softmax, and
            # the c-weighted s products all run chunk-by-chunk so DVE streams
            # without inter-pass barriers; PE and ACT ride behind.
            for it in range(1, ITERS):
                sp = spsum.tile([128, 512], F32, tag="sp", name=f"sp_{it}")

                def agree_mult(ch):
                    sl = slice(ch * CH, ch * CH + CH)
                    pa = work.tile([128, CH, O, C], BF16, tag="prod",
                                   name=f"pa{it}_{ch}")
                    nc.vector.tensor_mul(
                        pa[:], uhat[:, sl],
                        v_bf[:].unsqueeze(1).broadcast_to((128, CH, O, C)))
                    return pa

                # software pipeline: issue chunk ch+1's big multiply while
                # chunk ch's exp sits on ACT, so DVE never waits on ACT
                pa = agree_mult(0)
                for ch in range(NCH):
                    g0 = ch * CH
                    sl = slice(g0, g0 + CH)
                    nc.vector.tensor_add(pa[:, :, 0:8], pa[:, :, 0:8],
                                         pa[:, :, 8:16])
                    nc.vector.tensor_add(pa[:, :, 0:4], pa[:, :, 0:4],
                                         pa[:, :, 4:8])
                    nc.vector.tensor_add(pa[:, :, 0:2], pa[:, :, 0:2],
                                         pa[:, :, 2:4])
                    if it == 1:
                        nc.vector.tensor_add(bij[:, sl],
                                             pa[:, :, 0], pa[:, :, 1])
                    else:
                        nc.vector.tensor_add(pa[:, :, 0], pa[:, :, 0],
                                             pa[:, :, 1])
                        nc.vector.tensor_add(bij[:, sl],
                                             bij[:, sl], pa[:, :, 0])
                    # chunk-local softmax over c (exp on ACT)
                    nc.scalar.activation(cexp[:, sl], bij[:, sl],
                                         mybir.ActivationFunctionType.Exp)
                    pa_next = agree_mult(ch + 1) if ch + 1 < NCH else None
                    nc.vector.reduce_sum(zsum[:, sl], cexp[:, sl],
                                         axis=mybir.AxisListType.X)
                    nc.vector.reciprocal(zsum[:, sl], zsum[:, sl])
                    nc.vector.tensor_mul(
                        cbf[:, sl, 0:C], cexp[:, sl],
                        zsum[:, sl].unsqueeze(2).broadcast_to((128, CH, C)))
                    # s products + PE group-triple sums
                    prods = work.tile([128, CH, O, C], BF16, tag="prod",
                                      name=f"psx{it}_{ch}")
                    nc.vector.tensor_mul(
                        prods[:], uhat[:, sl],
                        cbf[:, sl, 0:C].unsqueeze(2)
                        .broadcast_to((128, CH, O, C)))
                    # 10 triples + one pair per 32-group chunk
                    for j in range(10):
                        nc.tensor.matmul(
                            sp[:, 0:480], lhsT=bmask[:],
                            rhs=prods[:, 3 * j : 3 * j + 3].rearrange(
                                "p g o c -> p (g o c)"),
                            start=(ch == 0 and j == 0), stop=False)
                    nc.tensor.matmul(
                        sp[:, 0 : 2 * CO], lhsT=bmask[:],
                        rhs=prods[:, 30:32].rearrange("p g o c -> p (g o c)"),
                        start=False, stop=(ch == NCH - 1))
                    pa = pa_next
                _s_combine(nc, sp, s_sb, 1.0)
                _squash(nc, work, s_sb, sq, sq2, v_bf, eps_t)

            # ---------- output ----------
            vfin = work.tile([128, O, C], F32, tag="vfin")
            nc.vector.tensor_mul(
                vfin[:], s_sb[:],
                sq[:].unsqueeze(1).broadcast_to((128, O, C)))
            nc.sync.dma_start(vout_d[:], vfin[0:B])

    nc.compile()
    return nc


def _s_combine(nc, sp, s_sb, scale):
    # s_sb = (sp[0:160] + sp[160:320] + sp[320:480]) * scale
    f = s_sb.rearrange("p o c -> p (o c)")
    nc.scalar.copy(f, sp[:, 0:CO])
    nc.vector.tensor_add(f, f, sp[:, CO : 2 * CO])
    nc.vector.tensor_add(f, f, sp[:, 2 * CO : 3 * CO])
    if scale != 1.0:
        nc.scalar.mul(f, f, scale)


def _squash(nc, work, s_sb, sq, sq2, v16, eps_t):
    """v = s * (|s|^2/(1+|s|^2)) / sqrt(|s|^2 + 1e-8), per (b, c).

    Leaves the scale factor in `sq`; v16 = s * scale (bf16).
    s_sb layout (B, O, C).
    """
    P = s_sb.shape[0]
    ssq = work.tile([P, O, C], F32, tag="ssq")
    nc.vector.tensor_mul(ssq[:], s_sb[:], s_sb[:])
    nc.vector.reduce_sum(sq[:], ssq[:].rearrange("p o c -> p c o"),
                         axis=mybir.AxisListType.X)
    # sq2 = (1+n)*sqrt(n+1e-8);  sq = n / sq2
    nc.scalar.activation(sq2[:], sq[:], mybir.ActivationFunctionType.Sqrt,
                         bias=eps_t[0:P])
    nc.vector.scalar_tensor_tensor(
        sq2[:], sq[:], 1.0, sq2[:],
        op0=mybir.AluOpType.add, op1=mybir.AluOpType.mult)
    nc.vector.reciprocal(sq2[:], sq2[:])
    nc.vector.tensor_mul(sq[:], sq[:], sq2[:])
    nc.vector.tensor_mul(
        v16[:], s_sb[:], sq[:].unsqueeze(1).broadcast_to((P, O, C)))


def kernel(x, W):
    global _COMPILED
    in_maps = _host_prep(x, W)
    if _COMPILED is None:
        _COMPILED = _build_kernel()
    res = run_bass_kernel_spmd(_COMPILED, in_maps, list(range(N_CORES)))
    outs = []
    for ci in range(N_CORES):
        v = res.results[ci]["vout"]  # (16, O, C)
        outs.append(v.transpose(0, 2, 1))  # -> (16, C, O)
    return np.ascontiguousarray(np.concatenate(outs, axis=0), dtype=np.float32)


# revision 33
# speedup vs baseline: 1.1145x; 1.1145x over previous
"""DigitCaps dynamic-routing kernel for 8x Trainium2 NeuronCores.

Full inputs -> batch-sharded across 8 cores (16 samples/core), W replicated.

Per-core layout:
  u_hat[(r_l,b)=128 partitions, g=256, o=16, c=10]  (bf16 in SBUF)
    where route r = g*8 + r_l   (8 routes per matmul group)
  b_ij / c_ij: (128p, 256g, 10c)   (partition carries (r_l, b))

u_hat build (one K=128 matmul per PAIR of groups):
  lhsT = xstk[pair]  (K=128 = two stacked 64-row x-blocks, M=128=(r_l,b))
  rhs  = wd[pair]    (K=128, N=320) block-diagonal W: rows 0:64 feed cols
         0:160 (group 2m), rows 64:128 feed cols 160:320 (group 2m+1);
         zero blocks memset once in two ping-pong SBUF slots, the DMA
         rewrites only the same diagonal windows each reuse.
  out  = psum (128, 320) -> 2 MMs share a 2-bank psum tile, drained as one
         FD=640 cast to bf16, alternating scalar/vector engines.

Iteration-1 sums (c == 0.1 exactly) are PE matmuls with lhsT =
tile(eye(16),(8,8)) interleaved into the phase-1 instruction stream so the
PE queue stays dense (sums over partitions (r_l) while replicating the
result to all 128 partitions).

Iterations 2..3: fused per-chunk pipeline: agreement = DVE broadcast
multiply + tree-reduce over o, chunk-local softmax over c (exp on ACT),
c-weighted products, PE group-triple sums.
"""

import sys

for p in ("/opt/trn_rl_repo",):
    if p not in sys.path:
        sys.path.insert(0, p)

import numpy as np
import ml_dtypes

import concourse.bass as bass
import concourse.bacc as bacc
import concourse.mybir as mybir
import concourse.tile as tile
from concourse.bass_utils import run_bass_kernel_spmd

# Problem constants (hardcoded per contract)
B_FULL = 128
N_CORES = 8
B = B_FULL // N_CORES  # 16 samples per core
R = 2048
C = 10
O = 16
I = 8
ITERS = 3

RG = 8               # routes per matmul group
G = R // RG          # 256 groups
K = RG * I           # 64 contraction rows per group
CO = C * O           # 160
NP = G // 2          # 128 group pairs
SC = 8               # pairs per phase-1 slot chunk
NSC = NP // SC       # 16 slot chunks
CH = 32              # groups per routing chunk
NCH = G // CH        # 8 chunks
CPAD = 12            # padded capsule dim for 4B alignment of bf16 rows

F32 = mybir.dt.float32
BF16 = mybir.dt.bfloat16

_COMPILED = None  # cache nc across calls


def _host_prep(x, W):
    """Build per-core DMA-ready arrays. x: (128,2048,8) W: (2048,10,16,8)."""
    x = np.ascontiguousarray(x, dtype=np.float32)
    W = np.ascontiguousarray(W, dtype=np.float32)

    # Wt[g, r_l*8+i, o*10+c] = W[g*8+r_l, c, o, i]
    Wt = W.transpose(0, 3, 2, 1).reshape(G, RG, I, O, C).reshape(G, K, CO)
    # wsrc[64*gam + krow, pair, n] = Wt[2*pair+gam, krow, n]
    wsrc = np.ascontiguousarray(
        Wt.reshape(NP, 2, K, CO).transpose(1, 2, 0, 3).reshape(2 * K, NP, CO)
    ).astype(ml_dtypes.bfloat16)

    # Bmask[(r_l,b), (r_l',b')] = 1 if b==b'  -> psum = sum over r_l,
    # replicated across all output partitions
    bmask = np.tile(np.eye(B, dtype=np.float32), (RG, RG)).astype(
        ml_dtypes.bfloat16)

    in_maps = []
    for ci in range(N_CORES):
        xs = x[ci * B : (ci + 1) * B]  # (16, 2048, 8)
        # xt[g, r_l, i, b] = xs[b, g*8+r_l, i]
        xt = xs.transpose(1, 2, 0).reshape(G, RG, I, B)
        # Block-diagonal lhsT: xblk[g, r_l*8+i, r_l*16+b] = xt[g, r_l, i, b]
        xblk = np.zeros((G, RG, I, RG, B), dtype=np.float32)
        idx = np.arange(RG)
        xblk[:, idx, :, idx, :] = xt.transpose(1, 0, 2, 3)
        xblk = xblk.reshape(G, K, RG * B)
        # xstk[64*gam + krow, pair, m] = xblk[2*pair+gam, krow, m]
        xstk = np.ascontiguousarray(
            xblk.reshape(NP, 2, K, RG * B).transpose(1, 2, 0, 3)
            .reshape(2 * K, NP, RG * B)
        ).astype(ml_dtypes.bfloat16)
        # Dense x for the direct s1 = 0.1*sum_{r,i} W*x matmuls:
        # xd[64*gam + 8*rl + i, q, b] = x[b, 16*q + 8*gam + rl, i]
        xd = np.ascontiguousarray(
            xt.reshape(NP, 2, RG, I, B).transpose(1, 2, 3, 0, 4)
            .reshape(2 * K, NP, B)
        ).astype(ml_dtypes.bfloat16)
        in_maps.append({"xstk": xstk, "xd": xd, "wsrc": wsrc,
                        "bmask": bmask})
    return in_maps


def _build_kernel():
    nc = bacc.Bacc("TRN2", target_bir_lowering=False, debug=False,
                   num_devices=N_CORES)

    xstk_d = nc.dram_tensor("xstk", [2 * K, NP, RG * B], BF16,
                            kind="ExternalInput")
    xd_d = nc.dram_tensor("xd", [2 * K, NP, B], BF16, kind="ExternalInput")
    wsrc_d = nc.dram_tensor("wsrc", [2 * K, NP, CO], BF16,
                            kind="ExternalInput")
    bmask_d = nc.dram_tensor("bmask", [128, 128], BF16, kind="ExternalInput")
    vout_d = nc.dram_tensor("vout", [B, O, C], F32, kind="ExternalOutput")

    NWD = 4  # wd slot count: 2-chunk DMA lookahead during the fused phase

    with tile.TileContext(nc) as tc:
        with (
            tc.tile_pool(name="persist", bufs=1) as persist,
            tc.tile_pool(name="xw", bufs=4) as xw,
            tc.tile_pool(name="work", bufs=7) as work,
            tc.tile_pool(name="psum", bufs=3, space="PSUM") as psum,
            tc.tile_pool(name="spsum", bufs=2, space="PSUM") as spsum,
        ):
            uhat = persist.tile([128, G, O, C], BF16)      # 80 KiB/part
            wd = [persist.tile([128, SC, 2 * CO], BF16, name=f"wd{s}")
                  for s in range(NWD)]                     # 4x 5 KiB
            wden = [persist.tile([128, SC, CO], BF16, name=f"wden{s}")
                    for s in range(NWD)]                   # 4x 2.5 KiB
            xdt = persist.tile([128, NP, B], BF16)         # 4 KiB
            bij = persist.tile([128, G, C], BF16)          # 5 KiB
            cexp = persist.tile([128, G, C], F32)          # 10 KiB
            cbf = persist.tile([128, G, CPAD], BF16)       # 6 KiB
            bmask = persist.tile([128, 128], BF16)
            v_bf = persist.tile([128, O, C], BF16)
            s_sb = persist.tile([128, O, C], F32)
            sq = persist.tile([128, C], F32)
            sq2 = persist.tile([128, C], F32)
            zsum = persist.tile([128, G], F32)
            eps_t = persist.tile([128, 1], F32)
            nc.gpsimd.memset(eps_t[:], 1e-8)
            for s in range(NWD):
                nc.gpsimd.memset(wd[s][:], 0.0)
            nc.sync.dma_start(bmask[:], bmask_d[:])
            nc.sync.dma_start(xdt[:], xd_d[:])

            # ---------- Phase A: iteration-1 sums direct from x, W -------
            # s1 = 0.1 * sum_r u_hat (c == 0.1 exactly) accumulates
            # directly as s1[b,(o,c)] = 0.1 * sum_{r,i} x[b,(r,i)]
            # W[(r,i),(o,c)]: one K=128 matmul per 16-route chunk with a
            # 16-column stationary (xd) against dense W slots. This gives
            # v1 ~16us in -- long before u_hat exists -- so the u_hat
            # build can then overlap iteration 2.
            sp1 = spsum.tile([128, 512], F32, tag="sp", name="sp1")
            # Prime the PE's HAM clock gate: ~3us of matmul-busy on the
            # zeroed wd[1] slot flips the PE to 2.4 GHz before the real
            # matmuls start (LDWEIGHTS does not count as PE-busy).
            for pw in range(10):
                nc.tensor.matmul(
                    sp1[:, 0 : 2 * CO], lhsT=bmask[:], rhs=wd[1][:, 0],
                    start=True, stop=True, skip_group_check=True)
            for sc in range(NSC):
                wn = wden[sc % NWD]
                eng = nc.sync if sc % 2 == 0 else nc.gpsimd
                eng.dma_start(wn[:], wsrc_d[:, sc * SC : (sc + 1) * SC])
                for j in range(SC):
                    q = sc * SC + j
                    nc.tensor.matmul(
                        sp1[0:B, 0:CO], lhsT=xdt[:, q], rhs=wn[:, j],
                        start=(q == 0), stop=(q == NP - 1),
                        skip_group_check=True)
            # s1 = 0.1 * sp1; squash on the 16 valid partitions, then
            # replicate v to all 128 partitions with a bmask matmul
            nc.scalar.activation(
                s_sb[0:B].rearrange("p o c -> p (o c)"), sp1[0:B, 0:CO],
                mybir.ActivationFunctionType.Copy, scale=0.1)
            _squash(nc, work, s_sb[0:B], sq[0:B], sq2[0:B], v_bf[0:B],
                    eps_t)
            vps = spsum.tile([128, 512], F32, tag="sp", name="vps")
            nc.tensor.matmul(
                vps[:, 0:CO], lhsT=bmask[0:B],
                rhs=v_bf[0:B].rearrange("p o c -> p (o c)"),
                start=True, stop=True)
            nc.vector.tensor_copy(
                v_bf[:].rearrange("p o c -> p (o c)"), vps[:, 0:CO])

            # ---------- Phase B: u_hat build, fused into iteration 2 -----
            # One K=128 matmul per pair of groups; 2 MMs share a 2-bank
            # psum tile drained as a single FD=640 cast. Slots 0..3 are
            # built up front; slots 4..15 are embedded two-chunks-ahead
            # inside the iteration-2 loop so the PE/ACT u_hat work hides
            # under the DVE-bound routing chunks.
            def b_slot_dma(sc):
                w = wd[sc % NWD]
                nc.sync.dma_start(
                    w[0:K, :, 0:CO], wsrc_d[0:K, sc * SC : (sc + 1) * SC])
                nc.gpsimd.dma_start(
                    w[K : 2 * K, :, CO : 2 * CO],
                    wsrc_d[K : 2 * K, sc * SC : (sc + 1) * SC])
                xt = xw.tile([128, SC, RG * B], BF16, tag="xt",
                             name=f"xt{sc}")
                eng = nc.sync if sc % 2 == 0 else nc.gpsimd
                eng.dma_start(xt[:], xstk_d[:, sc * SC : (sc + 1) * SC])
                return xt

            def b_slot_compute(sc, xt, drain_engs):
                w = wd[sc % NWD]
                for jj in range(SC // 2):
                    ps = psum.tile([128, 2, 512], F32, tag="ps",
                                   name=f"ps{sc}_{jj}")
                    for u in range(2):
                        j = 2 * jj + u
                        nc.tensor.matmul(
                            ps[:, u, 0 : 2 * CO],
                            lhsT=xt[:, j], rhs=w[:, j],
                            start=True, stop=True, skip_group_check=True)
                    g0 = sc * 2 * SC + 4 * jj
                    dst = uhat[:, g0 : g0 + 4].rearrange(
                        "p (x a) o c -> p x (a o c)", x=2)
                    src = ps[:, :, 0 : 2 * CO]
                    if drain_engs[jj % len(drain_engs)] == "act":
                        nc.scalar.copy(dst, src)
                    else:
                        nc.vector.tensor_copy(dst, src)

            xts = {}
            for sc in range(4):
                xts[sc] = b_slot_dma(sc)
            for sc in range(4):
                b_slot_compute(sc, xts.pop(sc), ("act", "dve"))
                if sc + 4 < NSC:
                    xts[sc + 4] = b_slot_dma(sc + 4)

            # ---------- iterations 2..3 ----------
            # Fused per-chunk pipeline: agreement, chunk-local softmax, and
            # the c-weighted s products all run chunk-by-chunk so DVE
            # streams without inter-pass barriers; PE and ACT ride behind.
            # During iteration 2, u_hat slots (2ch+4, 2ch+5) are built two
            # chunks ahead of their consumer, drained on ACT only (DVE
            # stays clean for the routing math).
            for it in range(1, ITERS):
                sp = spsum.tile([128, 512], F32, tag="sp", name=f"sp_{it}")

                def agree_mult(ch):
                    sl = slice(ch * CH, ch * CH + CH)
                    pa = work.tile([128, CH, O, C], BF16, tag="prod",
                                   name=f"pa{it}_{ch}")
                    nc.vector.tensor_mul(
                        pa[:], uhat[:, sl],
                        v_bf[:].unsqueeze(1).broadcast_to((128, CH, O, C)))
                    return pa

                # software pipeline: issue chunk ch+1's big multiply while
                # chunk ch's exp sits on ACT, so DVE never waits on ACT
                pa = agree_mult(0)
                for ch in range(NCH):
                    g0 = ch * CH
                    sl = slice(g0, g0 + CH)
                    nc.vector.tensor_add(pa[:, :, 0:8], pa[:, :, 0:8],
                                         pa[:, :, 8:16])
                    nc.vector.tensor_add(pa[:, :, 0:4], pa[:, :, 0:4],
                                         pa[:, :, 4:8])
                    nc.vector.tensor_add(pa[:, :, 0:2], pa[:, :, 0:2],
                                         pa[:, :, 2:4])
                    if it == 1:
                        nc.vector.tensor_add(bij[:, sl],
                                             pa[:, :, 0], pa[:, :, 1])
                    else:
                        nc.vector.tensor_add(pa[:, :, 0], pa[:, :, 0],
                                             pa[:, :, 1])
                        nc.vector.tensor_add(bij[:, sl],
                                             bij[:, sl], pa[:, :, 0])
                    # chunk-local softmax over c (exp on ACT)
                    nc.scalar.activation(cexp[:, sl], bij[:, sl],
                                         mybir.ActivationFunctionType.Exp)
                    # embedded u_hat build: slots for chunk ch+2's window,
                    # plus just-in-time DMA issue for slots 4 ahead (the
                    # wd/xt buffers are 4-deep; a DMA issued before its
                    # slot's previous consumer would read stale data)
                    if it == 1:
                        for s2 in (2 * ch + 4, 2 * ch + 5):
                            if s2 < NSC:
                                b_slot_compute(s2, xts.pop(s2), ("act",))
                                if s2 + 4 < NSC:
                                    xts[s2 + 4] = b_slot_dma(s2 + 4)
                    pa_next = agree_mult(ch + 1) if ch + 1 < NCH else None
                    nc.vector.reduce_sum(zsum[:, sl], cexp[:, sl],
                                         axis=mybir.AxisListType.X)
                    nc.vector.reciprocal(zsum[:, sl], zsum[:, sl])
                    nc.vector.tensor_mul(
                        cbf[:, sl, 0:C], cexp[:, sl],
                        zsum[:, sl].unsqueeze(2).broadcast_to((128, CH, C)))
                    # s products + PE group-triple sums
                    prods = work.tile([128, CH, O, C], BF16, tag="prod",
                                      name=f"psx{it}_{ch}")
                    nc.vector.tensor_mul(
                        prods[:], uhat[:, sl],
                        cbf[:, sl, 0:C].unsqueeze(2)
                        .broadcast_to((128, CH, O, C)))
                    # 10 triples + one pair per 32-group chunk
                    for j in range(10):
                        nc.tensor.matmul(
                            sp[:, 0:480], lhsT=bmask[:],
                            rhs=prods[:, 3 * j : 3 * j + 3].rearrange(
                                "p g o c -> p (g o c)"),
                            start=(ch == 0 and j == 0), stop=False,
                            skip_group_check=True)
                    nc.tensor.matmul(
                        sp[:, 0 : 2 * CO], lhsT=bmask[:],
                        rhs=prods[:, 30:32].rearrange("p g o c -> p (g o c)"),
                        start=False, stop=(ch == NCH - 1),
                        skip_group_check=True)
                    pa = pa_next
                _s_combine(nc, sp, s_sb, 1.0)
                _squash(nc, work, s_sb, sq, sq2, v_bf, eps_t)

            # ---------- output ----------
            vfin = work.tile([128, O, C], F32, tag="vfin")
            nc.vector.tensor_mul(
                vfin[:], s_sb[:],
                sq[:].unsqueeze(1).broadcast_to((128, O, C)))
            nc.sync.dma_start(vout_d[:], vfin[0:B])

    nc.compile()
    return nc


def _s_combine(nc, sp, s_sb, scale):
    # s_sb = (sp[0:160] + sp[160:320] + sp[320:480]) * scale
    f = s_sb.rearrange("p o c -> p (o c)")
    nc.scalar.copy(f, sp[:, 0:CO])
    nc.vector.tensor_add(f, f, sp[:, CO : 2 * CO])
    nc.vector.tensor_add(f, f, sp[:, 2 * CO : 3 * CO])
    if scale != 1.0:
        nc.scalar.mul(f, f, scale)


def _squash(nc, work, s_sb, sq, sq2, v16, eps_t):
    """v = s * (|s|^2/(1+|s|^2)) / sqrt(|s|^2 + 1e-8), per (b, c).

    Leaves the scale factor in `sq`; v16 = s * scale (bf16).
    s_sb layout (B, O, C).
    """
    P = s_sb.shape[0]
    ssq = work.tile([P, O, C], F32, tag="ssq")
    nc.vector.tensor_mul(ssq[:], s_sb[:], s_sb[:])
    nc.vector.reduce_sum(sq[:], ssq[:].rearrange("p o c -> p c o"),
                         axis=mybir.AxisListType.X)
    # sq2 = (1+n)*sqrt(n+1e-8);  sq = n / sq2
    nc.scalar.activation(sq2[:], sq[:], mybir.ActivationFunctionType.Sqrt,
                         bias=eps_t[0:P])
    nc.vector.scalar_tensor_tensor(
        sq2[:], sq[:], 1.0, sq2[:],
        op0=mybir.AluOpType.add, op1=mybir.AluOpType.mult)
    nc.vector.reciprocal(sq2[:], sq2[:])
    nc.vector.tensor_mul(sq[:], sq[:], sq2[:])
    nc.vector.tensor_mul(
        v16[:], s_sb[:], sq[:].unsqueeze(1).broadcast_to((P, O, C)))


def kernel(x, W):
    global _COMPILED
    in_maps = _host_prep(x, W)
    if _COMPILED is None:
        _COMPILED = _build_kernel()
    res = run_bass_kernel_spmd(_COMPILED, in_maps, list(range(N_CORES)))
    outs = []
    for ci in range(N_CORES):
        v = res.results[ci]["vout"]  # (16, O, C)
        outs.append(v.transpose(0, 2, 1))  # -> (16, C, O)
    return np.ascontiguousarray(np.concatenate(outs, axis=0), dtype=np.float32)


# revision 34
# speedup vs baseline: 1.3923x; 1.2493x over previous
"""DigitCaps dynamic-routing kernel for 8x Trainium2 NeuronCores.

Full inputs -> batch-sharded across 8 cores (16 samples/core), W replicated.

Per-core layout:
  u_hat[(r_l,b)=128 partitions, g=256, o=16, c=10]  (bf16 in SBUF)
    where route r = g*8 + r_l   (8 routes per matmul group)
  b_ij / c_ij: (128p, 256g, 10c)   (partition carries (r_l, b))

u_hat build (one K=128 matmul per PAIR of groups):
  lhsT = xstk[pair]  (K=128 = two stacked 64-row x-blocks, M=128=(r_l,b))
  rhs  = wd[pair]    (K=128, N=320) block-diagonal W: rows 0:64 feed cols
         0:160 (group 2m), rows 64:128 feed cols 160:320 (group 2m+1);
         zero blocks memset once in two ping-pong SBUF slots, the DMA
         rewrites only the same diagonal windows each reuse.
  out  = psum (128, 320) -> 2 MMs share a 2-bank psum tile, drained as one
         FD=640 cast to bf16, alternating scalar/vector engines.

Iteration-1 sums (c == 0.1 exactly) are PE matmuls with lhsT =
tile(eye(16),(8,8)) interleaved into the phase-1 instruction stream so the
PE queue stays dense (sums over partitions (r_l) while replicating the
result to all 128 partitions).

Iterations 2..3: fused per-chunk pipeline: agreement = DVE broadcast
multiply + tree-reduce over o, chunk-local softmax over c (exp on ACT),
c-weighted products, PE group-triple sums.
"""

import sys

for p in ("/opt/trn_rl_repo",):
    if p not in sys.path:
        sys.path.insert(0, p)

import numpy as np
import ml_dtypes

import concourse.bass as bass
import concourse.bacc as bacc
import concourse.mybir as mybir
import concourse.tile as tile
from concourse.bass_utils import run_bass_kernel_spmd

# Problem constants (hardcoded per contract)
B_FULL = 128
N_CORES = 8
B = B_FULL // N_CORES  # 16 samples per core
R = 2048
C = 10
O = 16
I = 8
ITERS = 3

RG = 8               # routes per matmul group
G = R // RG          # 256 groups
K = RG * I           # 64 contraction rows per group
CO = C * O           # 160
NP = G // 2          # 128 group pairs
SC = 8               # pairs per phase-1 slot chunk
NSC = NP // SC       # 16 slot chunks
CH = 32              # groups per routing chunk
NCH = G // CH        # 8 chunks
CPAD = 12            # padded capsule dim for 4B alignment of bf16 rows

F32 = mybir.dt.float32
BF16 = mybir.dt.bfloat16

_COMPILED = None  # cache nc across calls


def _host_prep(x, W):
    """Build per-core DMA-ready arrays. x: (128,2048,8) W: (2048,10,16,8)."""
    x = np.ascontiguousarray(x, dtype=np.float32)
    W = np.ascontiguousarray(W, dtype=np.float32)

    # Wt[g, r_l*8+i, o*10+c] = W[g*8+r_l, c, o, i]
    Wt = W.transpose(0, 3, 2, 1).reshape(G, RG, I, O, C).reshape(G, K, CO)
    # wsrc[64*gam + krow, pair, n] = Wt[2*pair+gam, krow, n]
    wsrc = np.ascontiguousarray(
        Wt.reshape(NP, 2, K, CO).transpose(1, 2, 0, 3).reshape(2 * K, NP, CO)
    ).astype(ml_dtypes.bfloat16)

    # Bmask[(r_l,b), (r_l',b')] = 1 if b==b'  -> psum = sum over r_l,
    # replicated across all output partitions
    bmask = np.tile(np.eye(B, dtype=np.float32), (RG, RG)).astype(
        ml_dtypes.bfloat16)

    in_maps = []
    for ci in range(N_CORES):
        xs = x[ci * B : (ci + 1) * B]  # (16, 2048, 8)
        # xt[g, r_l, i, b] = xs[b, g*8+r_l, i]
        xt = xs.transpose(1, 2, 0).reshape(G, RG, I, B)
        # Block-diagonal lhsT: xblk[g, r_l*8+i, r_l*16+b] = xt[g, r_l, i, b]
        xblk = np.zeros((G, RG, I, RG, B), dtype=np.float32)
        idx = np.arange(RG)
        xblk[:, idx, :, idx, :] = xt.transpose(1, 0, 2, 3)
        xblk = xblk.reshape(G, K, RG * B)
        # xstk[64*gam + krow, pair, m] = xblk[2*pair+gam, krow, m]
        xstk = np.ascontiguousarray(
            xblk.reshape(NP, 2, K, RG * B).transpose(1, 2, 0, 3)
            .reshape(2 * K, NP, RG * B)
        ).astype(ml_dtypes.bfloat16)
        # Dense x for the direct s1 = 0.1*sum_{r,i} W*x matmuls:
        # xd[64*gam + 8*rl + i, q, b] = x[b, 16*q + 8*gam + rl, i]
        xd = np.ascontiguousarray(
            xt.reshape(NP, 2, RG, I, B).transpose(1, 2, 3, 0, 4)
            .reshape(2 * K, NP, B)
        ).astype(ml_dtypes.bfloat16)
        in_maps.append({"xstk": xstk, "xd": xd, "wsrc": wsrc,
                        "bmask": bmask})
    return in_maps


def _build_kernel():
    nc = bacc.Bacc("TRN2", target_bir_lowering=False, debug=False,
                   num_devices=N_CORES)

    xstk_d = nc.dram_tensor("xstk", [2 * K, NP, RG * B], BF16,
                            kind="ExternalInput")
    xd_d = nc.dram_tensor("xd", [2 * K, NP, B], BF16, kind="ExternalInput")
    wsrc_d = nc.dram_tensor("wsrc", [2 * K, NP, CO], BF16,
                            kind="ExternalInput")
    bmask_d = nc.dram_tensor("bmask", [128, 128], BF16, kind="ExternalInput")
    vout_d = nc.dram_tensor("vout", [B, O, C], F32, kind="ExternalOutput")

    NWD = 4  # wd slot count: 2-chunk DMA lookahead during the fused phase

    with tile.TileContext(nc) as tc:
        with (
            tc.tile_pool(name="persist", bufs=1) as persist,
            tc.tile_pool(name="xw", bufs=4) as xw,
            tc.tile_pool(name="work", bufs=5) as work,
            tc.tile_pool(name="psum", bufs=3, space="PSUM") as psum,
            tc.tile_pool(name="spsum", bufs=2, space="PSUM") as spsum,
        ):
            uhat = persist.tile([128, G, O, C], BF16)      # 80 KiB/part
            wd = [persist.tile([128, SC, 2 * CO], BF16, name=f"wd{s}")
                  for s in range(NWD)]                     # 4x 5 KiB
            wden = [persist.tile([128, SC, CO], BF16, name=f"wden{s}")
                    for s in range(NWD)]                   # 4x 2.5 KiB
            xdt = persist.tile([128, NP, B], BF16)         # 4 KiB
            bij = persist.tile([128, G, C], BF16)          # 5 KiB
            cexp = persist.tile([128, G, C], F32)          # 10 KiB
            cbf = persist.tile([128, G, CPAD], BF16)       # 6 KiB
            bmask = persist.tile([128, 128], BF16)
            v_bf = persist.tile([128, O, C], BF16)
            s_sb = persist.tile([128, O, C], F32)
            sq = persist.tile([128, C], F32)
            sq2 = persist.tile([128, C], F32)
            zsum = persist.tile([128, G], F32)
            eps_t = persist.tile([128, 1], F32)
            nc.gpsimd.memset(eps_t[:], 1e-8)
            for s in range(NWD):
                nc.gpsimd.memset(wd[s][:], 0.0)
            nc.sync.dma_start(bmask[:], bmask_d[:])
            nc.sync.dma_start(xdt[:], xd_d[:])

            # ---------- Phase A: iteration-1 sums direct from x, W -------
            # s1 = 0.1 * sum_r u_hat (c == 0.1 exactly) accumulates
            # directly as s1[b,(o,c)] = 0.1 * sum_{r,i} x[b,(r,i)]
            # W[(r,i),(o,c)]: one K=128 matmul per 16-route chunk with a
            # 16-column stationary (xd) against dense W slots. This gives
            # v1 ~16us in -- long before u_hat exists -- so the u_hat
            # build can then overlap iteration 2.
            sp1 = spsum.tile([128, 512], F32, tag="sp", name="sp1")
            # Prime the PE's HAM clock gate: ~3us of matmul-busy on the
            # zeroed wd[1] slot flips the PE to 2.4 GHz before the real
            # matmuls start (LDWEIGHTS does not count as PE-busy).
            for pw in range(10):
                nc.tensor.matmul(
                    sp1[:, 0 : 2 * CO], lhsT=bmask[:], rhs=wd[1][:, 0],
                    start=True, stop=True, skip_group_check=True)
            for sc in range(NSC):
                wn = wden[sc % NWD]
                eng = nc.sync if sc % 2 == 0 else nc.gpsimd
                eng.dma_start(wn[:], wsrc_d[:, sc * SC : (sc + 1) * SC])
                for j in range(SC):
                    q = sc * SC + j
                    nc.tensor.matmul(
                        sp1[0:B, 0:CO], lhsT=xdt[:, q], rhs=wn[:, j],
                        start=(q == 0), stop=(q == NP - 1),
                        skip_group_check=True)
            # s1 = 0.1 * sp1; squash on the 16 valid partitions, then
            # replicate v to all 128 partitions with a bmask matmul
            nc.scalar.activation(
                s_sb[0:B].rearrange("p o c -> p (o c)"), sp1[0:B, 0:CO],
                mybir.ActivationFunctionType.Copy, scale=0.1)
            _squash(nc, work, s_sb[0:B], sq[0:B], sq2[0:B], v_bf[0:B],
                    eps_t)
            vps = spsum.tile([128, 512], F32, tag="sp", name="vps")
            nc.tensor.matmul(
                vps[:, 0:CO], lhsT=bmask[0:B],
                rhs=v_bf[0:B].rearrange("p o c -> p (o c)"),
                start=True, stop=True)
            nc.vector.tensor_copy(
                v_bf[:].rearrange("p o c -> p (o c)"), vps[:, 0:CO])

            # ---------- Phase B: u_hat build, fused into iteration 2 -----
            # One K=128 matmul per pair of groups; 2 MMs share a 2-bank
            # psum tile drained as a single FD=640 cast. Slots 0..3 are
            # built up front; slots 4..15 are embedded two-chunks-ahead
            # inside the iteration-2 loop so the PE/ACT u_hat work hides
            # under the DVE-bound routing chunks.
            def b_slot_dma(sc):
                w = wd[sc % NWD]
                nc.sync.dma_start(
                    w[0:K, :, 0:CO], wsrc_d[0:K, sc * SC : (sc + 1) * SC])
                nc.gpsimd.dma_start(
                    w[K : 2 * K, :, CO : 2 * CO],
                    wsrc_d[K : 2 * K, sc * SC : (sc + 1) * SC])
                xt = xw.tile([128, SC, RG * B], BF16, tag="xt",
                             name=f"xt{sc}")
                eng = nc.sync if sc % 2 == 0 else nc.gpsimd
                eng.dma_start(xt[:], xstk_d[:, sc * SC : (sc + 1) * SC])
                return xt

            def b_slot_compute(sc, xt, drain_engs):
                w = wd[sc % NWD]
                for jj in range(SC // 2):
                    ps = psum.tile([128, 2, 512], F32, tag="ps",
                                   name=f"ps{sc}_{jj}")
                    for u in range(2):
                        j = 2 * jj + u
                        nc.tensor.matmul(
                            ps[:, u, 0 : 2 * CO],
                            lhsT=xt[:, j], rhs=w[:, j],
                            start=True, stop=True, skip_group_check=True)
                    g0 = sc * 2 * SC + 4 * jj
                    dst = uhat[:, g0 : g0 + 4].rearrange(
                        "p (x a) o c -> p x (a o c)", x=2)
                    src = ps[:, :, 0 : 2 * CO]
                    if drain_engs[jj % len(drain_engs)] == "act":
                        nc.scalar.copy(dst, src)
                    else:
                        nc.vector.tensor_copy(dst, src)

            xts = {}
            for sc in range(4):
                xts[sc] = b_slot_dma(sc)
            for sc in range(4):
                b_slot_compute(sc, xts.pop(sc), ("act", "dve"))
                if sc + 4 < NSC:
                    xts[sc + 4] = b_slot_dma(sc + 4)

            # ---------- iterations 2..3 ----------
            # Fused per-chunk pipeline: agreement, chunk-local softmax, and
            # the c-weighted s products all run chunk-by-chunk so DVE
            # streams without inter-pass barriers; PE and ACT ride behind.
            # During iteration 2, u_hat slots (2ch+4, 2ch+5) are built two
            # chunks ahead of their consumer, drained on ACT only (DVE
            # stays clean for the routing math).
            for it in range(1, ITERS):
                sp = spsum.tile([128, 512], F32, tag="sp", name=f"sp_{it}")

                def agree_mult(ch):
                    sl = slice(ch * CH, ch * CH + CH)
                    pa = work.tile([128, CH, O, C], BF16, tag="prod",
                                   name=f"pa{it}_{ch}")
                    nc.vector.tensor_mul(
                        pa[:], uhat[:, sl],
                        v_bf[:].unsqueeze(1).broadcast_to((128, CH, O, C)))
                    return pa

                # software pipeline: issue chunk ch+1's big multiply while
                # chunk ch's exp sits on ACT, so DVE never waits on ACT
                pa = agree_mult(0)
                for ch in range(NCH):
                    g0 = ch * CH
                    sl = slice(g0, g0 + CH)
                    nc.vector.tensor_add(pa[:, :, 0:8], pa[:, :, 0:8],
                                         pa[:, :, 8:16])
                    nc.vector.tensor_add(pa[:, :, 0:4], pa[:, :, 0:4],
                                         pa[:, :, 4:8])
                    nc.vector.tensor_add(pa[:, :, 0:2], pa[:, :, 0:2],
                                         pa[:, :, 2:4])
                    if it == 1:
                        nc.vector.tensor_add(bij[:, sl],
                                             pa[:, :, 0], pa[:, :, 1])
                    else:
                        nc.vector.tensor_add(pa[:, :, 0], pa[:, :, 0],
                                             pa[:, :, 1])
                        nc.vector.tensor_add(bij[:, sl],
                                             bij[:, sl], pa[:, :, 0])
                    # chunk-local softmax over c (exp on ACT)
                    nc.scalar.activation(cexp[:, sl], bij[:, sl],
                                         mybir.ActivationFunctionType.Exp)
                    # embedded u_hat build: slots for chunk ch+2's window,
                    # plus just-in-time DMA issue for slots 4 ahead (the
                    # wd/xt buffers are 4-deep; a DMA issued before its
                    # slot's previous consumer would read stale data)
                    if it == 1:
                        for s2 in (2 * ch + 4, 2 * ch + 5):
                            if s2 < NSC:
                                b_slot_compute(s2, xts.pop(s2), ("act",))
                                if s2 + 4 < NSC:
                                    xts[s2 + 4] = b_slot_dma(s2 + 4)
                    pa_next = agree_mult(ch + 1) if ch + 1 < NCH else None
                    nc.vector.reduce_sum(zsum[:, sl], cexp[:, sl],
                                         axis=mybir.AxisListType.X)
                    nc.vector.reciprocal(zsum[:, sl], zsum[:, sl])
                    nc.vector.tensor_mul(
                        cbf[:, sl, 0:C], cexp[:, sl],
                        zsum[:, sl].unsqueeze(2).broadcast_to((128, CH, C)))
                    # s products + PE group-triple sums
                    prods = work.tile([128, CH, O, C], BF16, tag="prod",
                                      name=f"psx{it}_{ch}")
                    nc.vector.tensor_mul(
                        prods[:], uhat[:, sl],
                        cbf[:, sl, 0:C].unsqueeze(2)
                        .broadcast_to((128, CH, O, C)))
                    # 10 triples + one pair per 32-group chunk
                    for j in range(10):
                        nc.tensor.matmul(
                            sp[:, 0:480], lhsT=bmask[:],
                            rhs=prods[:, 3 * j : 3 * j + 3].rearrange(
                                "p g o c -> p (g o c)"),
                            start=(ch == 0 and j == 0), stop=False,
                            skip_group_check=True)
                    nc.tensor.matmul(
                        sp[:, 0 : 2 * CO], lhsT=bmask[:],
                        rhs=prods[:, 30:32].rearrange("p g o c -> p (g o c)"),
                        start=False, stop=(ch == NCH - 1),
                        skip_group_check=True)
                    pa = pa_next
                _s_combine(nc, sp, s_sb, 1.0)
                _squash(nc, work, s_sb, sq, sq2, v_bf, eps_t)

            # ---------- output ----------
            vfin = work.tile([128, O, C], F32, tag="vfin")
            nc.vector.tensor_mul(
                vfin[:], s_sb[:],
                sq[:].unsqueeze(1).broadcast_to((128, O, C)))
            nc.sync.dma_start(vout_d[:], vfin[0:B])

    nc.compile()
    return nc


def _s_combine(nc, sp, s_sb, scale):
    # s_sb = (sp[0:160] + sp[160:320] + sp[320:480]) * scale
    f = s_sb.rearrange("p o c -> p (o c)")
    nc.scalar.copy(f, sp[:, 0:CO])
    nc.vector.tensor_add(f, f, sp[:, CO : 2 * CO])
    nc.vector.tensor_add(f, f, sp[:, 2 * CO : 3 * CO])
    if scale != 1.0:
        nc.scalar.mul(f, f, scale)


def _squash(nc, work, s_sb, sq, sq2, v16, eps_t):
    """v = s * (|s|^2/(1+|s|^2)) / sqrt(|s|^2 + 1e-8), per (b, c).

    Leaves the scale factor in `sq`; v16 = s * scale (bf16).
    s_sb layout (B, O, C).
    """
    P = s_sb.shape[0]
    ssq = work.tile([P, O, C], F32, tag="ssq")
    nc.vector.tensor_mul(ssq[:], s_sb[:], s_sb[:])
    nc.vector.reduce_sum(sq[:], ssq[:].rearrange("p o c -> p c o"),
                         axis=mybir.AxisListType.X)
    # sq2 = (1+n)*sqrt(n+1e-8);  sq = n / sq2
    nc.scalar.activation(sq2[:], sq[:], mybir.ActivationFunctionType.Sqrt,
                         bias=eps_t[0:P])
    nc.vector.scalar_tensor_tensor(
        sq2[:], sq[:], 1.0, sq2[:],
        op0=mybir.AluOpType.add, op1=mybir.AluOpType.mult)
    nc.vector.reciprocal(sq2[:], sq2[:])
    nc.vector.tensor_mul(sq[:], sq[:], sq2[:])
    nc.vector.tensor_mul(
        v16[:], s_sb[:], sq[:].unsqueeze(1).broadcast_to((P, O, C)))


def kernel(x, W):
    global _COMPILED
    in_maps = _host_prep(x, W)
    if _COMPILED is None:
        _COMPILED = _build_kernel()
    res = run_bass_kernel_spmd(_COMPILED, in_maps, list(range(N_CORES)))
    outs = []
    for ci in range(N_CORES):
        v = res.results[ci]["vout"]  # (16, O, C)
        outs.append(v.transpose(0, 2, 1))  # -> (16, C, O)
    return np.ascontiguousarray(np.concatenate(outs, axis=0), dtype=np.float32)
